# revision 65
# baseline (speedup 1.0000x reference)
"""Hausdorff distance kernel for Trainium2 (8 NeuronCores).

Reference computes, per sample n (N=2), on a 20^3 voxel grid (V=8000):
  d[i,j]   = Euclidean distance between voxel centers (coords / 20)
  min_to_B = min over j in B of d[i,j]
  distA    = max over i in Aonly of min_to_B   (Aonly = A & ~B)
  (symmetrically distB), haus_n = max(distA, distB); output = mean_n haus_n.

Strategy (separable Gaussian-sum distance field):
  On the integer grid, md2[p] = min_{q in B} |p-q|^2 is an integer. With
  S[p] = sum_{q in B} exp(-a*|p-q|^2) and a=10, we get
      -ln(S[p])/a = md2[p] - ln(n0 + eps)/a,   n0 = #minimizers <= 24,
  so round(-ln(S)/a) recovers md2 EXACTLY while S stays in fp32 range
  (md2 <= 8; larger values underflow detectably -> exact host fallback,
  which cannot trigger unless the Hausdorff distance exceeds sqrt(8)
  voxels).  exp(-a*|p-q|^2) factors over axes, so S = (Kx x Ky x Kz) * B
  is computed with 4 small matmuls per (sample, direction):
    stage 1:  out1[x,(y',z')] = sum_{(y,z)} B[(y,z),x] * K2[(y,z),(y',z')]
              (3 PSUM-accumulated matmuls over the y-groups within
              |dy|<=2 of this half; the 4th group's K2 is ~0 in bf16)
    stage 2:  S[x',(y',z')]   = sum_x Kx[x,x'] * out1[x,(y',z')]
  The row mask rides into PSUM through the same stage-2 matmul (an
  identity block stacked under Kx with the 0/3e38 mask stacked under
  out1), so one DVE free-axis min-reduce finishes the directed distance;
  the host takes the final 20-way min / sqrt / mean on scalars.
  8 cores = 4 (sample,direction) problems x 2 halves of the (y',z')
  output space; per-core time is dominated by fixed costs (BSP preamble
  ~7us, input-DMA latency ~2.8us, output-DMA completion ~1.9us), the
  compute chain itself is ~2us.
"""

import sys
import functools

import numpy as np

for _p in ("/opt/trn_rl_repo",):
    if _p not in sys.path:
        sys.path.insert(0, _p)

import ml_dtypes  # noqa: E402
from concourse import bass, mybir  # noqa: E402
from concourse.bass_utils import run_bass_kernel_spmd  # noqa: E402

D = H = W = 20
V = D * H * W
N_CORES = 8
BIG = 1e9
ALPHA = 10.0
S_FLOOR = 1e-36  # S below this => md2 >= 9 possible => exact fallback
MASK_BIG = 3.0e38
F32 = mybir.dt.float32
BF16 = mybir.dt.bfloat16
NPBF16 = ml_dtypes.bfloat16

# kin layout (bf16, [100, 880]); only the 3 y-groups within |dy|<=2 of the
# core's (y',z') half contribute (the 4th K2 chunk is ~exp(-360), i.e. 0):
#   chunk c in {0,1,2}: cols [220c, 220c+220): B-mask [:, 0:20), K2 [:, 20:220)
#   [0:40, 660:680)  stacked stationary [Kx ; I] for fused stage2+mask matmul
#   [0:20, 680:880)  row mask as 0.0 / MASK_BIG for this (y',z') half
#                    (re-DMA'd into partitions 20:40 of the rhs2 tensor)
KIN_COLS = 880
KIN_ROWS = 100


@functools.lru_cache(maxsize=None)
def _kernels():
    yy, zz = np.meshgrid(np.arange(H), np.arange(W), indexing="ij")
    yz = np.stack([yy, zz], -1).reshape(400, 2).astype(np.float64)
    k2 = np.exp(-ALPHA * ((yz[:, None, :] - yz[None, :, :]) ** 2).sum(-1))
    kx = np.exp(-ALPHA * (np.subtract.outer(np.arange(D), np.arange(D)) ** 2.0))
    return k2.astype(NPBF16), kx.astype(NPBF16)


@functools.lru_cache(maxsize=None)
def _build():
    """Per-core program: S-field for one (problem, half) -> masked min scalar."""
    # Skip bass's end-of-init all-engine barrier and pseudo-sync barrier
    # (~1.4us of preamble): they only order the gpsimd sem_clear/dma_reset
    # (done by ~2us) against later semaphore use, and this kernel's first
    # semaphore increment is a DMA completion that lands >2us after that.
    # All cross-engine ordering below flows through explicit semaphores.
    _orig_aeb = bass.Bass.all_engine_barrier
    _orig_npb = bass.Bass._nrt_pseudo_barrier
    bass.Bass.all_engine_barrier = lambda self, *, sem_only=False: None
    bass.Bass._nrt_pseudo_barrier = lambda self: None
    try:
        nc = bass.Bass()
    finally:
        bass.Bass.all_engine_barrier = _orig_aeb
        bass.Bass._nrt_pseudo_barrier = _orig_npb
    kin_d = nc.declare_dram_parameter("kin", [KIN_ROWS, KIN_COLS], BF16, isOutput=False)
    out_d = nc.declare_dram_parameter("out", [20, 1], F32, isOutput=True)

    with (
        nc.sbuf_tensor("kin_t", [KIN_ROWS, KIN_COLS], BF16) as kin_t,
        nc.sbuf_tensor("rhs2", [40, 200], BF16) as rhs2,
        nc.sbuf_tensor("red32", [20, 1], F32) as red32,
        nc.psum_tensor("ps1", [20, 200], F32) as ps1,
        nc.psum_tensor("ps2", [20, 200], F32) as ps2,
        nc.psum_tensor("warm", [20, 512], F32) as warm,
        nc.semaphore("ina_sem") as ina_sem,
        nc.semaphore("inb_sem") as inb_sem,
        nc.semaphore("inc_sem") as inc_sem,
        nc.semaphore("pe_sem") as pe_sem,
        nc.semaphore("dve_sem") as dve_sem,
        nc.semaphore("out_sem") as out_sem,
    ):
        # inputs pipelined across both HWDGE queues: chunks 0-1 on sync,
        # chunk 2 on scalar (kept small: the scalar queue's issue is ~700ns
        # slower and gates mm3), stationary on sync, mask replica on scalar
        nc.sync.dma_start(
            out=kin_t.ap()[:, 0:440], in_=kin_d[:, 0:440], single_packet=True
        ).then_inc(ina_sem, 16)
        nc.scalar.dma_start(out=kin_t.ap()[:, 440:660], in_=kin_d[:, 440:660]).then_inc(
            inb_sem, 16
        )
        nc.sync.dma_start(
            out=kin_t.ap()[0:40, 660:680], in_=kin_d[0:40, 660:680]
        ).then_inc(inc_sem, 16)
        nc.scalar.dma_start(out=rhs2.ap()[20:40, :], in_=kin_d[0:20, 680:880]).then_inc(
            inc_sem, 16
        )


        # PE warmup during the DMA wait: read garbage SBUF into a scratch
        # PSUM bank; results never consumed. Keeps the PE pipeline out of
        # its cold p-state for the real matmuls.
        for n_mov in (512, 512, 512, 512, 512, 256):
            nc.tensor.matmul(
                warm.ap()[:, 0:n_mov],
                kin_t.ap()[0:20, 0:20],
                kin_t.ap()[0:20, 0:n_mov],
                start=True,
                stop=True,
            )

        # stage 1: 3 accumulating matmuls over the relevant (y,z) chunks
        nc.tensor.wait_ge(ina_sem, 16)
        for c in range(2):
            nc.tensor.matmul(
                ps1.ap(),
                kin_t.ap()[:, 220 * c : 220 * c + 20],
                kin_t.ap()[:, 220 * c + 20 : 220 * (c + 1)],
                start=(c == 0),
                stop=False,
            )
        nc.tensor.wait_ge(inb_sem, 16)
        nc.tensor.matmul(
            ps1.ap(), kin_t.ap()[:, 440:460], kin_t.ap()[:, 460:660],
            start=False, stop=True,
        ).then_inc(pe_sem, 1)

        # PSUM -> SBUF bf16 into rhs2[0:20] (DVE, not Scalar: scalar.copy
        # would trigger a ~2.3us ACT_TABLE_LOAD in the hot path)
        nc.vector.wait_ge(pe_sem, 1)
        nc.vector.tensor_scalar_add(rhs2.ap()[0:20, :], ps1.ap(), 0.0).then_inc(
            dve_sem, 1
        )

        # fused stage 2: [Kx ; I]^T @ [out1 ; mask] contracts x AND adds the
        # row mask in a single matmul
        nc.tensor.wait_ge(inc_sem, 32)
        nc.tensor.wait_ge(dve_sem, 1)
        nc.tensor.matmul(
            ps2.ap(), kin_t.ap()[0:40, 660:680], rhs2.ap(), start=True, stop=True
        ).then_inc(pe_sem, 1)

        # masked min over free dim straight from PSUM
        nc.vector.wait_ge(pe_sem, 2)
        nc.vector.tensor_reduce(
            red32.ap()[0:20, 0:1],
            ps2.ap(),
            axis=mybir.AxisListType.X,
            op=mybir.AluOpType.min,
        ).then_inc(dve_sem, 1)

        nc.sync.wait_ge(dve_sem, 2)
        # no explicit wait on out_sem: the BSP epilogue's queue drain
        # already blocks NEFF completion on the DMA landing in DRAM, so the
        # epilogue overlaps the DMA-completion latency instead of
        # serializing after it
        nc.sync.dma_start(
            out=out_d[:], in_=red32.ap()[0:20, 0:1], single_packet=True
        ).then_inc(out_sem, 16)
    return nc


def _make_kin(rows_mask, cols_mask, half):
    """Build the [100, 880] bf16 input for one (problem, half)."""
    k2, kx = _kernels()
    b3 = cols_mask.reshape(D, H, W)
    byz_x = np.ascontiguousarray(b3.transpose(1, 2, 0).reshape(400, D)).astype(NPBF16)
    rows3 = rows_mask.reshape(D, H, W)
    maskbig = np.where(
        rows3[:, 10 * half : 10 * (half + 1), :].reshape(D, 200), 0.0, MASK_BIG
    ).astype(NPBF16)

    kin = np.zeros((KIN_ROWS, KIN_COLS), NPBF16)
    # y-groups of 5 within |dy|<=2 of this half's y' range; the dropped
    # group's K2 entries are <= exp(-10*9) ~ 0 in bf16
    groups = (0, 1, 2) if half == 0 else (1, 2, 3)
    for i, g in enumerate(groups):
        kin[:, 220 * i : 220 * i + 20] = byz_x[100 * g : 100 * (g + 1)]
        kin[:, 220 * i + 20 : 220 * (i + 1)] = k2[
            100 * g : 100 * (g + 1), 200 * half : 200 * (half + 1)
        ]
    kin[0:20, 660:680] = kx
    kin[20:40, 660:680] = np.eye(20, dtype=NPBF16)
    kin[0:20, 680:880] = maskbig
    return kin


def _exact_md2max(rows_mask, cols_mask):
    """Host fallback: exact max-min squared distance (integer grid)."""
    x, y, z = np.meshgrid(np.arange(D), np.arange(H), np.arange(W), indexing="ij")
    coords = np.stack([x, y, z], -1).reshape(V, 3).astype(np.float64)
    rows = coords[rows_mask]
    cols = coords[cols_mask]
    best = 0.0
    for i in range(0, len(rows), 512):
        blk = rows[i : i + 512]
        d2 = ((blk[:, None, :] - cols[None, :, :]) ** 2).sum(-1)
        best = max(best, d2.min(axis=1).max())
    return best


def kernel(predict, target):
    predict = np.asarray(predict)
    target = np.asarray(target)
    n = predict.shape[0]
    im_a = np.round(predict.reshape(n, V)) != 0
    im_b = np.round(target.reshape(n, V)) != 0

    # 2*n directed problems: (rows = one-sided points, cols = other full set)
    probs = []
    for s in range(n):
        ma, mb = im_a[s], im_b[s]
        probs.append((ma & ~mb, mb))  # distA direction
        probs.append((mb & ~ma, ma))  # distB direction
    n_probs = len(probs)
    assert n_probs * 2 == N_CORES, "kernel hardcodes N=2 samples -> 4 problems"

    nc = _build()
    in_maps = []
    for c in range(N_CORES):
        rows_mask, cols_mask = probs[c // 2]
        if not rows_mask.any() or not cols_mask.any():
            # degenerate problem: feed a dummy; host overrides the result
            rows_mask = np.ones(V, bool)
            cols_mask = np.ones(V, bool)
        in_maps.append({"kin": _make_kin(rows_mask, cols_mask, c % 2)})
    results = run_bass_kernel_spmd(nc, in_maps, list(range(N_CORES))).results

    dists = np.empty(n_probs, np.float64)
    for p in range(n_probs):
        rows_mask, cols_mask = probs[p]
        if not rows_mask.any():
            dists[p] = 0.0
            continue
        if not cols_mask.any():
            dists[p] = BIG  # reference: min over empty B stays BIG
            continue
        s_min = min(
            float(np.asarray(results[2 * p]["out"]).reshape(-1)[:20].min()),
            float(np.asarray(results[2 * p + 1]["out"]).reshape(-1)[:20].min()),
        )
        est = -np.log(max(s_min, 1e-45)) / ALPHA if s_min > 0 else np.inf
        if s_min < S_FLOOR or abs(est - np.round(est)) > 0.4:
            md2 = _exact_md2max(rows_mask, cols_mask)  # device range exceeded
        else:
            md2 = float(np.round(est))
        dists[p] = np.sqrt(md2) / 20.0

    haus = np.empty(n, np.float64)
    for s in range(n):
        dist_a, dist_b = dists[2 * s], dists[2 * s + 1]
        ma, mb = im_a[s], im_b[s]
        if (mb & ~ma).any() and not ma.any():
            dist_b = 999.0
        haus[s] = max(dist_a, dist_b)
    return np.float32(haus.mean())
